# revision 2
# baseline (speedup 1.0000x reference)
"""GRU decoder kernel for Trainium2 (Bass/Tile), data-parallel over batch N.

Problem: T=1000, N=64, H=1024, C=256 GRU with batched input projection and
log_softmax output.  Each of the 8 cores handles N/8 = 8 batch rows:
  phase 1: xp = enc @ W_ih.T + biases   (bf16 matmul, PE-transposed enc)
  phase 2: 1000 sequential GRU steps    (bf16 W_hh stationary, h moving)
  phase 3: log_softmax over C           (PE transpose + Exp/Ln on ACT)

All phase-2 stream DMAs (xp ring refills, hist flushes) use layouts that
collapse to contiguous per-partition runs so descriptor counts stay low.
"""

import sys

for _p in ("/opt/trn_rl_repo",):
    if _p not in sys.path:
        sys.path.insert(0, _p)

import numpy as np
import ml_dtypes

import concourse.bass as bass
import concourse.bacc as bacc
import concourse.mybir as mybir
import concourse.tile as tile
from concourse.bass import ds, ts
from concourse.bass_utils import run_bass_kernel_spmd

F32 = mybir.dt.float32
BF16 = mybir.dt.bfloat16
AF = mybir.ActivationFunctionType
OP = mybir.AluOpType

T, N, H, C = 1000, 64, 1024, 256
G3 = 3 * C  # 768
NCORES = 8
NB = N // NCORES  # 8 batch rows per core
KH = H // 128     # 8 k-chunks for projection
M3 = G3 // 128    # 6 m-chunks of gate dim
KC = C // 128     # 2 k-chunks for recurrence
import os as _os
U = int(_os.environ.get("GRU_U", "32"))  # steps per For_i iteration
UH = U // 2


def build_gru_nc(t_total=T):
    """Build the Bass program. t_total must be = 8 (mod 16) or multiple of 16."""
    nc = bacc.Bacc(None, target_bir_lowering=False)

    rem = t_total % U
    n_iters = t_total // U
    assert rem == 0 or rem <= UH, f"t_total={t_total} bad for U={U}"

    # ---- parameters -----------------------------------------------------
    enc = nc.declare_dram_parameter("enc", [t_total * NB, H], F32, isOutput=False)
    # W_ih as lhsT tiles: wih[p, k, m*128+q] = W_ih[m*128+q, k*128+p]
    wih = nc.declare_dram_parameter("wih", [128, KH, G3], BF16, isOutput=False)
    # W_hh as lhsT tiles: whh[p, k, m*128+q] = W_hh[m*128+q, k*128+p]
    whh = nc.declare_dram_parameter("whh", [128, KC, G3], BF16, isOutput=False)
    idt_b = nc.declare_dram_parameter("idt_b", [128, 128], BF16, isOutput=False)
    idt_f = nc.declare_dram_parameter("idt_f", [128, 128], F32, isOutput=False)
    # bias column m: rz chunks get b_ih+b_hh, n chunks get b_ih only
    biasc = nc.declare_dram_parameter("biasc", [128, M3], F32, isOutput=False)
    # b_hh for the n gate, broadcast over batch: bhn[p, c2*8+j] = b_hh[2C+c2*128+p]
    bhn = nc.declare_dram_parameter("bhn", [128, 2 * NB], BF16, isOutput=False)
    out = nc.declare_dram_parameter("out", [t_total * NB, C], F32, isOutput=True)

    # ---- DRAM scratch (t-major, matching the SBUF ring layouts) ---------
    # pad t by UH: the lookahead refill of the last iteration overshoots
    xp_rz = nc.dram_tensor("xp_rz", [128, t_total + UH, 4, NB], BF16)
    xp_n = nc.dram_tensor("xp_n", [128, t_total + UH, 2, NB], F32)
    hist = nc.dram_tensor("hist", [128, 2, t_total, NB], F32)

    with tile.TileContext(nc) as tc:
        with (
            tc.tile_pool(name="const", bufs=1) as cpool,
            tc.tile_pool(name="work", bufs=2) as wpool,
        ):
            idt_b_sb = cpool.tile([128, 128], BF16)
            nc.sync.dma_start(idt_b_sb, idt_b[:, :])
            idt_f_sb = cpool.tile([128, 128], F32)
            nc.sync.dma_start(idt_f_sb, idt_f[:, :])
            wih_sb = cpool.tile([128, KH, G3], BF16)
            nc.sync.dma_start(wih_sb, wih[:, :, :])
            whh_sb = cpool.tile([128, KC, G3], BF16)
            nc.sync.dma_start(whh_sb, whh[:, :, :])
            biasc_sb = cpool.tile([128, M3], F32)
            nc.sync.dma_start(biasc_sb, biasc[:, :])
            bhn_sb = cpool.tile([128, 2 * NB], BF16)
            nc.sync.dma_start(bhn_sb, bhn[:, :])

            # ================= phase 1: input projection ================
            p1 = tc.tile_pool(name="p1psum", bufs=1, space="PSUM")
            pspool = p1.__enter__()
            p1t = tc.tile_pool(name="p1tpose", bufs=2, space="PSUM")
            ptpool = p1t.__enter__()
            t0 = 0
            while t0 < t_total:
                bt = min(64, t_total - t0)
                rows = bt * NB
                ntile = (rows + 127) // 128

                encT = wpool.tile([128, KH, 512], BF16, tag="encT")
                for ti in range(ntile):
                    r0 = t0 * NB + ti * 128
                    rr = min(128, t_total * NB - r0)
                    enc_sb = wpool.tile([128, H], F32, tag="enc_in", bufs=3)
                    nc.sync.dma_start(enc_sb[:rr, :], enc[ds(r0, rr), :])
                    enc_bf = wpool.tile([128, H], BF16, tag="enc_bf", bufs=2)
                    nc.scalar.activation(enc_bf[:rr, :], enc_sb[:rr, :], AF.Copy)
                    for k in range(KH):
                        ps_t = ptpool.tile([128, 128], BF16, tag="tpose")
                        nc.tensor.transpose(
                            ps_t[:, :rr], enc_bf[:rr, ts(k, 128)],
                            idt_b_sb[:rr, :rr],
                        )
                        nc.vector.tensor_copy(
                            encT[:, k, ds(ti * 128, rr)], ps_t[:, :rr]
                        )

                psm = [
                    pspool.tile(
                        [128, 512], F32, tag=f"pj_psum{m}", name=f"pj_psum{m}"
                    )
                    for m in range(M3)
                ]
                for k in range(KH):
                    for m in range(M3):
                        nc.tensor.matmul(
                            psm[m][:, :rows],
                            lhsT=wih_sb[:, k, ts(m, 128)],
                            rhs=encT[:, k, :rows],
                            start=(k == 0),
                            stop=(k == KH - 1),
                        )
                # stage into ring-layout tiles so the DRAM DMAs are contiguous
                st_rz = wpool.tile([128, 64, 4, NB], BF16, tag="st_rz", bufs=2)
                for m in range(4):
                    nc.vector.tensor_scalar(
                        st_rz[:, :bt, m, :],
                        psm[m][:, :rows].rearrange("p (t j) -> p t j", j=NB),
                        biasc_sb[:, m : m + 1], None, OP.add,
                    )
                nc.sync.dma_start(
                    xp_rz[:, ds(t0, bt), :, :], st_rz[:, :bt, :, :]
                )
                st_n = wpool.tile([128, 64, 2, NB], F32, tag="st_n", bufs=2)
                for m in (4, 5):
                    nc.vector.tensor_scalar(
                        st_n[:, :bt, m - 4, :],
                        psm[m][:, :rows].rearrange("p (t j) -> p t j", j=NB),
                        biasc_sb[:, m : m + 1], None, OP.add,
                    )
                nc.sync.dma_start(
                    xp_n[:, ds(t0, bt), :, :], st_n[:, :bt, :, :]
                )
                t0 += bt

            # zero the lookahead pad so the overshooting refill reads clean
            zpad = wpool.tile([128, UH, 4, NB], BF16, tag="zpad")
            nc.vector.memset(zpad, 0.0)
            nc.sync.dma_start(xp_rz[:, t_total : t_total + UH, :, :], zpad)
            zpad_n = wpool.tile([128, UH, 2, NB], F32, tag="zpad_n")
            nc.vector.memset(zpad_n, 0.0)
            nc.sync.dma_start(xp_n[:, t_total : t_total + UH, :, :], zpad_n)

            p1t.__exit__(None, None, None)
            p1.__exit__(None, None, None)

            # ================= phase 2: GRU recurrence ==================
            p2 = tc.tile_pool(name="p2psum", bufs=2, space="PSUM")
            ptpool = p2.__enter__()
            rz_ring = cpool.tile([128, U, 4, NB], BF16)
            n_ring = cpool.tile([128, U, 2, NB], F32)
            h_ring = cpool.tile([128, 2, U, NB], F32)   # fp32 h (c2-major)
            h_bf = cpool.tile([128, 2, KC * NB], BF16)  # ping-pong bf16 h

            nc.vector.memset(h_bf[:, :, :], 0.0)
            nc.gpsimd.memset(h_ring[:, :, U - 1, :], 0.0)

            # pre-warm the sigmoid/tanh table set so the body's table load
            # hoists out of the loop
            warm = wpool.tile([1, 1], F32, tag="warm")
            nc.scalar.activation(warm, bhn_sb[:1, :1], AF.Sigmoid)

            def emit_step(s_glob, slot):
                """One GRU step reading xp rings at `slot`, h from slot-1."""
                pv = (slot - 1) % U
                hb_in = h_bf[:, (s_glob + 1) % 2, :]
                hb_out = h_bf[:, s_glob % 2, :]
                hp3 = h_ring[:, :, pv, :]  # [128, 2, NB]
                v3 = lambda ap: ap.rearrange("p (c j) -> p c j", c=2)

                ps_r = ptpool.tile([128, 2 * NB], F32, tag="ps_r")
                ps_z = ptpool.tile([128, 2 * NB], F32, tag="ps_z")
                ps_n = ptpool.tile([128, 2 * NB], F32, tag="ps_n")

                # xp / bias preloads (independent of h): one group per tile
                nc.tensor.matmul(
                    ps_r, lhsT=idt_b_sb, rhs=rz_ring[:, slot, 0:2, :],
                    start=True, stop=False,
                )
                nc.tensor.matmul(
                    ps_z, lhsT=idt_b_sb, rhs=rz_ring[:, slot, 2:4, :],
                    start=True, stop=False,
                )
                nc.tensor.matmul(
                    ps_n, lhsT=idt_b_sb, rhs=bhn_sb, start=True, stop=False,
                )

                # W_hh matmuls: r chunks, then n, then z.  stop only on the
                # very last matmul touching each psum tile (zero-region rule).
                def wmm(ps, m, col, last):
                    for k in range(KC):
                        nc.tensor.matmul(
                            ps[:, ts(col, NB)],
                            lhsT=whh_sb[:, k, ts(m, 128)],
                            rhs=hb_in[:, ts(k, NB)],
                            start=False,
                            stop=(last and k == KC - 1),
                        )

                for ps, ms in ((ps_r, (0, 1)), (ps_n, (4, 5)), (ps_z, (2, 3))):
                    for c2, m in enumerate(ms):
                        wmm(ps, m, c2, c2 == 1)

                r_sb = wpool.tile([128, 2 * NB], F32, tag="r_sb")
                nc.scalar.activation(r_sb, ps_r, AF.Sigmoid)
                zb_sb = wpool.tile([128, 2 * NB], F32, tag="zb_sb")
                nc.scalar.activation(zb_sb, ps_z, AF.Sigmoid, scale=-1.0)

                m1 = wpool.tile([128, 2 * NB], F32, tag="m1")
                nc.vector.tensor_tensor(m1, r_sb, ps_n, OP.mult)
                nin = wpool.tile([128, 2 * NB], F32, tag="nin")
                nc.vector.tensor_tensor(
                    v3(nin), v3(m1), n_ring[:, slot, :, :], OP.add,
                )
                n_sb = wpool.tile([128, 2 * NB], F32, tag="n_sb")
                nc.scalar.activation(n_sb, nin, AF.Tanh)

                q_sb = wpool.tile([128, 2 * NB], F32, tag="q_sb")
                nc.gpsimd.tensor_tensor(v3(q_sb), v3(zb_sb), hp3, OP.mult)
                t2 = wpool.tile([128, 2 * NB], F32, tag="t2")
                nc.gpsimd.tensor_tensor(v3(t2), hp3, v3(q_sb), OP.subtract)
                zn = wpool.tile([128, 2 * NB], F32, tag="zn")
                nc.vector.tensor_tensor(zn, zb_sb, n_sb, OP.mult)
                nc.vector.tensor_tensor(hb_out, zn, t2, OP.add)
                nc.gpsimd.tensor_tensor(
                    h_ring[:, :, slot, :], v3(zn), v3(t2), OP.add,
                )

            def refill(iv, lo, hi):
                nc.sync.dma_start(
                    rz_ring[:, lo:hi, :, :], xp_rz[:, ds(iv, hi - lo), :, :]
                )
                nc.sync.dma_start(
                    n_ring[:, lo:hi, :, :], xp_n[:, ds(iv, hi - lo), :, :]
                )

            def flush(iv, lo, hi):
                for c2 in range(KC):
                    nc.sync.dma_start(
                        hist[:, c2, ds(iv, hi - lo), :],
                        h_ring[:, c2, lo:hi, :],
                    )

            refill(0, 0, UH)  # prologue: slots 0..7 <- t 0..7
            if n_iters > 0:
                with tc.For_i(
                    0, n_iters * U, step=U, staggered_reset=True,
                    hint_engines=(mybir.EngineType.PE,),
                ) as iv:
                    refill(iv + UH, UH, U)
                    for s in range(UH):
                        emit_step(s, s)
                    flush(iv, 0, UH)
                    refill(iv + U, 0, UH)
                    for s in range(UH, U):
                        emit_step(s, s)
                    flush(iv + UH, UH, U)
            if rem:
                base = n_iters * U
                for s in range(rem):
                    emit_step(s, s)
                flush(base, 0, rem)

            p2.__exit__(None, None, None)

            # ================= phase 3: log_softmax =====================
            p3 = tc.tile_pool(name="p3psum", bufs=2, space="PSUM")
            ptpool = p3.__enter__()
            nblk = (t_total + 15) // 16
            se_all = cpool.tile([128, nblk], F32)
            mx_all = cpool.tile([128, nblk], F32)
            nc.vector.memset(se_all, 1.0)
            nc.vector.memset(mx_all, 0.0)

            def p3_transpose(t0, bt):
                rows = bt * NB
                hsb = wpool.tile([128, 2, 16, NB], F32, tag="hsb", bufs=3)
                nc.sync.dma_start(hsb[:, :, :bt, :], hist[:, :, ds(t0, bt), :])
                ps3 = ptpool.tile([128, 256], F32, tag="ps3")
                for c2 in range(KC):
                    nc.tensor.transpose(
                        ps3[:rows, ts(c2, 128)], hsb[:, c2, :bt, :], idt_f_sb
                    )
                return ps3, rows

            # pass 1: max + sum(exp(x-max)) per block (Exp table only)
            blocks = []
            t0 = 0
            b = 0
            while t0 < t_total:
                bt = min(16, t_total - t0)
                blocks.append((t0, bt, b))
                ps3, rows = p3_transpose(t0, bt)
                nc.vector.tensor_reduce(
                    mx_all[:rows, b : b + 1], ps3[:rows, :],
                    mybir.AxisListType.X, OP.max,
                )
                ngm = wpool.tile([128, 1], F32, tag="ngm")
                nc.vector.tensor_scalar_mul(
                    ngm[:rows, :], mx_all[:rows, b : b + 1], -1.0
                )
                escr = wpool.tile([128, 256], BF16, tag="escr")
                nc.scalar.activation(
                    escr[:rows, :], ps3[:rows, :], AF.Exp,
                    bias=ngm[:rows, :], accum_out=se_all[:rows, b : b + 1],
                )
                t0 += bt
                b += 1

            # one Ln over all blocks, then s = mx + ln(se)
            lz_all = cpool.tile([128, nblk], F32)
            nc.scalar.activation(lz_all, se_all, AF.Ln)
            s_all = cpool.tile([128, nblk], F32)
            nc.vector.tensor_tensor(s_all, mx_all, lz_all, OP.add)

            # pass 2: out = x - s
            for t0, bt, b in blocks:
                ps3, rows = p3_transpose(t0, bt)
                o_sb = wpool.tile([128, 256], F32, tag="o_sb", bufs=3)
                nc.vector.tensor_scalar(
                    o_sb[:rows, :], ps3[:rows, :], s_all[:rows, b : b + 1],
                    None, OP.subtract,
                )
                nc.sync.dma_start(out[ds(t0 * NB, rows), :], o_sb[:rows, :])
            p3.__exit__(None, None, None)

    nc.compile()
    return nc


def _prep_weights(W_ih, W_hh, b_ih, b_hh):
    bf = ml_dtypes.bfloat16
    # lhsT layouts: w[p, k, m*128+q] = W[m*128+q, k*128+p]
    def lhst(W, kc):
        t = W.T.reshape(kc, 128, W.shape[0])  # [k, p, g]
        return np.ascontiguousarray(t.transpose(1, 0, 2)).astype(bf)

    wih = lhst(W_ih, KH)
    whh = lhst(W_hh, KC)
    idt_b = np.eye(128, dtype=bf)
    idt_f = np.eye(128, dtype=np.float32)
    ball = (b_ih + b_hh).astype(np.float32).copy()
    ball[2 * C :] = b_ih[2 * C :]  # n gate: b_ih only (b_hh_n goes inside r*)
    biasc = np.ascontiguousarray(ball.reshape(M3, 128).T).astype(np.float32)
    bhn_v = b_hh[2 * C :].reshape(2, 128).T  # [p, c2]
    bhn = np.repeat(bhn_v[:, :, None], NB, axis=2).reshape(128, 2 * NB).astype(bf)
    return wih, whh, idt_b, idt_f, biasc, bhn


_CACHED = {}


def _make_in_maps(inputs):
    encoder_output = np.asarray(inputs["encoder_output"], dtype=np.float32)
    W_ih = np.asarray(inputs["W_ih"], dtype=np.float32)
    W_hh = np.asarray(inputs["W_hh"], dtype=np.float32)
    b_ih = np.asarray(inputs["b_ih"], dtype=np.float32)
    b_hh = np.asarray(inputs["b_hh"], dtype=np.float32)
    t_total = encoder_output.shape[0]
    wih, whh, idt_b, idt_f, biasc, bhn = _prep_weights(W_ih, W_hh, b_ih, b_hh)
    in_maps = []
    for c in range(NCORES):
        shard = encoder_output[:, c * NB : (c + 1) * NB, :]  # [T, NB, H]
        in_maps.append(
            {
                "enc": np.ascontiguousarray(shard.reshape(t_total * NB, H)),
                "wih": wih, "whh": whh, "idt_b": idt_b, "idt_f": idt_f,
                "biasc": biasc, "bhn": bhn,
            }
        )
    return in_maps


def kernel(encoder_output, W_ih, W_hh, b_ih, b_hh):
    encoder_output = np.asarray(encoder_output, dtype=np.float32)

    t_total = encoder_output.shape[0]
    if "nc" not in _CACHED or _CACHED.get("t") != t_total:
        _CACHED["nc"] = build_gru_nc(t_total)
        _CACHED["t"] = t_total

    in_maps = _make_in_maps(
        {
            "encoder_output": encoder_output,
            "W_ih": W_ih, "W_hh": W_hh, "b_ih": b_ih, "b_hh": b_hh,
        }
    )

    res = run_bass_kernel_spmd(_CACHED["nc"], in_maps, list(range(NCORES)))
    outs = [
        res.results[c]["out"].reshape(t_total, NB, C) for c in range(NCORES)
    ]
    return np.concatenate(outs, axis=1)


if __name__ == "__main__":
    # quick smoke test with small T
    t_small = 24
    rng = np.random.default_rng(0)
    enc = rng.standard_normal((t_small, N, H), dtype=np.float32)
    s = 0.05
    Wih = rng.standard_normal((G3, H), dtype=np.float32) * s
    Whh = rng.standard_normal((G3, C), dtype=np.float32) * s
    bih = rng.standard_normal(G3).astype(np.float32) * s
    bhh = rng.standard_normal(G3).astype(np.float32) * s

    got = kernel(enc, Wih, Whh, bih, bhh)

    xp = enc.reshape(-1, H) @ Wih.T + bih
    xp = xp.reshape(t_small, N, G3)
    h = np.zeros((N, C), dtype=np.float32)
    outs = []
    sig = lambda x: 1.0 / (1.0 + np.exp(-x))
    for t in range(t_small):
        gh = h @ Whh.T + bhh
        xr, xz, xn = np.split(xp[t], 3, axis=-1)
        hr, hz, hn = np.split(gh, 3, axis=-1)
        r = sig(xr + hr)
        z = sig(xz + hz)
        n = np.tanh(xn + r * hn)
        h = (1.0 - z) * n + z * h
        outs.append(h.copy())
    ref = np.stack(outs)
    mx = ref.max(-1, keepdims=True)
    ref = ref - mx - np.log(np.exp(ref - mx).sum(-1, keepdims=True))

    err = np.abs(got - ref)
    print("abs err max:", err.max(), " rel:", err.max() / np.abs(ref).max())

